# revision 19
# baseline (speedup 1.0000x reference)
"""Trainium2 Bass kernel for nn_AffinityBiFC.

Reference computation (B=4, N=M=128, D=256, BD=1024):
    t  = einsum('bnd,dek->bnek', X, A)
    bi = einsum('bnek,bme->bnmk', t, Y)
    S  = einsum('bnmk,ok->bnmo', bi, W) + b        -> S[..., 0]  [B, N, M]

Algebraic collapse (exact reassociation):
    Aw[d, e] = sum_k A[d, e, k] * W[0, k]          # one streaming pass over A
    S[b]     = X[b] @ Aw @ Y[b].T + b              # tiny matmuls

Sharding: A is split over its first (d) axis across the 8 cores.  Each core
streams its 32 d-rows (16.75 MB as fp16), reduces them to Aw_c[32, 256],
computes its partial S_c = (X[:, :, rows_c] @ Aw_c) @ Y^T locally, and writes
S_c out.  The host sums the 8 partials and adds the bias — no device
collectives at all (the old AllGather-based design spent ~25us on the final
collective plus a ~48us serial tail).

Per-core pipeline (DMA-bound at ~350 GB/s for the 16.75 MB fp16 stream):
  - Host packs A_c as [kp=128, dl=32, kc=8, e=256] fp16 (k = kc*128 + kp), so
    k lives on SBUF partitions and each DMA group is 128 fully-contiguous
    per-partition runs.
  - DVE tensor_scalar multiplies each [128, r*256] block by W[kc*128+kp]
    (per-partition scalar -> eligible for the fast DVE modes).
  - PE reduces over the k partitions with an all-ones stationary operand
    (loaded once): psum[p, de] += sum_kp scr[kp, de]; 8 kc-blocks accumulate
    into one PSUM chunk.  All 128 psum rows are equal, so row 0 is the
    answer; tiny gpsimd DMAs stage it to DRAM (engines cannot write at a
    partition offset, so the d-partitioned Aw layout is rebuilt by one
    gather DMA instead).
  - Final: T = Aw_c^T X_c^T on PE, then S_c[b] = T^T Y_b^T, one fp32 copy,
    one 256 KB output DMA.
  - Numerics: products A*W would hit fp16 subnormals, so the host stages
    W*32 and X/32 (exact power-of-two rescale; S unchanged).
"""

import numpy as np

B, N, D, KD = 4, 128, 256, 1024
P = 128
C = 8                    # cores
DL = D // C              # 32 d-rows per core
KC = KD // P             # 8 k-blocks
# DMA groups paired into shared tiles so each DVE tensor_scalar spans up to
# 8 rows (2048 elems) instead of 4 — halves DVE op count at equal DMA grain
TILES = [[1, 1, 2], [4, 4], [4, 4], [4, 4], [2, 1, 1]]
assert sum(sum(tg) for tg in TILES) == DL
XSCALE = 32.0            # host stages W*32 and X/32 to keep A*W out of fp16 subnormals

_cached = {}


def _build_program():
    import concourse.bass as bass
    import concourse.mybir as mybir
    import concourse.tile as tile
    from concourse import bacc

    fp32 = mybir.dt.float32
    fp16 = mybir.dt.float16

    nc = bacc.Bacc(
        "TRN2",
        target_bir_lowering=False,
        debug=False,
        num_devices=C,
    )

    # host-packed A shard: [kp, dl, kc, e] fp16, k = kc*128 + kp
    a_sh = nc.dram_tensor("a_sh", [P, DL, KC, D], fp16, kind="ExternalInput").ap()
    w_in = nc.dram_tensor("w_in", [P, KC], fp32, kind="ExternalInput").ap()   # W[kc*128+kp]*32
    xt_in = nc.dram_tensor("xt_in", [DL, B, N], fp16, kind="ExternalInput").ap()  # (X/32)^T local rows
    yt_in = nc.dram_tensor("yt_in", [D, B, N], fp16, kind="ExternalInput").ap()   # Y^T [e, b, m]
    out = nc.dram_tensor("out", [B, N, N], fp32, kind="ExternalOutput").ap()
    DEBUG = _cached.get("debug", False)
    if DEBUG:
        dbg_ones = nc.dram_tensor("dbg_ones", [P, P], fp16, kind="ExternalOutput").ap()
        dbg_awflat = nc.dram_tensor("dbg_awflat", [1, DL * D], fp16, kind="ExternalOutput").ap()
        dbg_aw = nc.dram_tensor("dbg_aw", [DL, D], fp16, kind="ExternalOutput").ap()
        dbg_tT = nc.dram_tensor("dbg_tT", [P, 2, B, N], fp16, kind="ExternalOutput").ap()

    with tile.TileContext(nc) as tc:
        with (
            tc.tile_pool(name="apool", bufs=2) as apool,
            tc.tile_pool(name="spool", bufs=2) as spool,
            tc.tile_pool(name="sbuf", bufs=1) as sbuf,
            tc.tile_pool(name="pred", bufs=3, space="PSUM") as pred,
            tc.tile_pool(name="pfin", bufs=1, space="PSUM") as pfin,
            tc.tile_pool(name="dram", bufs=1, space="DRAM") as dram,
        ):
            # small inputs on the gpsimd SWDGE ring; sync ring stays on the A stream
            w_sb = sbuf.tile([P, KC], fp32)
            nc.gpsimd.dma_start(w_sb[:], w_in[:])
            xt_sb = sbuf.tile([DL, B, N], fp16)
            nc.gpsimd.dma_start(xt_sb[:], xt_in[:])
            yt_sb = sbuf.tile([P, 2, B, N], fp16)   # [e_lo, ec, b, m]
            nc.gpsimd.dma_start(yt_sb[:], yt_in.rearrange("(ec p) b m -> p ec b m", p=P))

            ones = sbuf.tile([P, P], fp16)
            nc.gpsimd.memset(ones[:], 1.0)

            aw_flat = sbuf.tile([1, DL * D], fp16)   # Aw staging on partition 0, (dl, e) order
            aw_sb = sbuf.tile([DL, D], fp16)

            r0 = 0
            for g, tile_groups in enumerate(TILES):
                tr = sum(tile_groups)
                at = apool.tile([P, 8, KC, D], fp16, tag="a", name=f"at{g}")
                off = 0
                for r in tile_groups:
                    nc.sync.dma_start(
                        at[:, off : off + r], a_sh[:, r0 + off : r0 + off + r]
                    )
                    off += r
                scr = spool.tile([P, 8, KC, D], fp16, tag="s", name=f"scr{g}")
                for kc in range(KC):
                    # scr = at * W[kc*128 + kp]  (per-partition scalar)
                    nc.vector.tensor_scalar_mul(
                        scr[:, :tr, kc], at[:, :tr, kc], w_sb[:, kc : kc + 1]
                    )
                # PE partition-reduce with all-ones stationary, 2 d-rows per chunk
                for c0 in range(0, tr, 2):
                    cw = min(2, tr - c0)
                    ps = pred.tile([P, 2 * D], fp32, tag="ps", name=f"ps{g}_{c0}")
                    for kc in range(KC):
                        nc.tensor.matmul(
                            ps[:, : cw * D],
                            lhsT=ones,
                            rhs=scr[:, c0 : c0 + cw, kc],
                            start=(kc == 0),
                            stop=(kc == KC - 1),
                        )
                    # all psum rows equal -> ACT stages row 0 (fp32->fp16 cast)
                    row = r0 + c0
                    nc.scalar.activation(
                        out=aw_flat[0:1, row * D : (row + cw) * D],
                        in_=ps[0:1, : cw * D],
                        func=mybir.ActivationFunctionType.Copy,
                    )
                r0 += tr

            # rebuild Aw with d on partitions.  An SBUF->SBUF partition
            # scatter miscompiles on HW (sim-only correct), so bounce
            # through DRAM: contiguous store, then the standard scattered load.
            aw_dram = dram.tile([1, DL * D], fp16)
            nc.gpsimd.dma_start(aw_dram[:], aw_flat[:])
            nc.gpsimd.dma_start(
                aw_sb[:], aw_dram.rearrange("o (r e) -> (o r) e", r=DL)
            )
            psT = [pfin.tile([P, B * N], fp32, name=f"psT{ec}") for ec in range(2)]
            for ec in range(2):
                nc.tensor.matmul(
                    psT[ec],
                    lhsT=aw_sb[:, ec * P : (ec + 1) * P],
                    rhs=xt_sb[:],
                    start=True,
                    stop=True,
                )
            tT = sbuf.tile([P, 2, B, N], fp16)   # [e_lo, ec, b, n]
            for ec in range(2):
                nc.scalar.activation(
                    out=tT[:, ec], in_=psT[ec][:, :],
                    func=mybir.ActivationFunctionType.Copy,
                )
            psS = pfin.tile([P, B, N], fp32)     # [n, b, m]
            for b in range(B):
                for ec in range(2):
                    nc.tensor.matmul(
                        psS[:, b, :],
                        lhsT=tT[:, ec, b, :],
                        rhs=yt_sb[:, ec, b, :],
                        start=(ec == 0),
                        stop=(ec == 1),
                    )
            s_sb = sbuf.tile([P, B, N], fp32)
            nc.scalar.activation(
                out=s_sb[:], in_=psS[:, :, :],
                func=mybir.ActivationFunctionType.Copy,
            )
            nc.sync.dma_start(out.rearrange("b n m -> n b m"), s_sb[:])
            if DEBUG:
                nc.sync.dma_start(dbg_ones[:], ones[:])
                nc.sync.dma_start(dbg_awflat[:], aw_flat[:])
                nc.sync.dma_start(dbg_aw[:], aw_sb[:])
                nc.sync.dma_start(dbg_tT[:], tT[:])

    nc.compile()
    return nc


def _get_program():
    if "nc" not in _cached:
        _cached["nc"] = _build_program()
    return _cached["nc"]


def _run(X, Y, A, W, b, trace=False, **trace_kwargs):
    from concourse.bass_utils import run_bass_kernel_spmd

    nc = _get_program()

    A = np.asarray(A, dtype=np.float32)
    W = np.asarray(W, dtype=np.float32)
    X = np.asarray(X, dtype=np.float32)
    Y = np.asarray(Y, dtype=np.float32)

    # W * 32 laid out [kp, kc]; X / 32 transposed to [d, b, n] (exact 2^5 rescale)
    w_cols = np.ascontiguousarray(
        (W.reshape(KC, P) * np.float32(XSCALE)).T, dtype=np.float32
    )
    xt = np.ascontiguousarray(
        (X / np.float32(XSCALE)).transpose(2, 0, 1), dtype=np.float16
    )  # [d, b, n]
    yt = np.ascontiguousarray(Y.transpose(2, 0, 1), dtype=np.float16)  # [e, b, m]

    in_maps = []
    for c in range(C):
        rows = slice(c * DL, (c + 1) * DL)
        # [dl, e, k] -> [kp, dl, kc, e]
        a_perm = np.ascontiguousarray(
            A[rows].reshape(DL, D, KC, P).transpose(3, 0, 2, 1), dtype=np.float16
        )
        in_maps.append(
            {
                "a_sh": a_perm,
                "w_in": w_cols,
                "xt_in": np.ascontiguousarray(xt[rows]),
                "yt_in": yt,
            }
        )

    res = run_bass_kernel_spmd(nc, in_maps, list(range(C)), trace=trace, **trace_kwargs)
    # per-core outputs are partial sums over d; host unshard = sum + bias
    out = np.zeros((B, N, N), dtype=np.float32)
    for c in range(C):
        out += np.asarray(res.results[c]["out"], dtype=np.float32)
    out += np.float32(np.asarray(b).reshape(-1)[0])
    return out, res


def kernel(X, Y, A, W, b):
    out, _ = _run(X, Y, A, W, b, trace=False)
    return out


# revision 20
# speedup vs baseline: 1.2756x; 1.2756x over previous
"""Trainium2 Bass kernel for nn_AffinityBiFC.

Reference computation (B=4, N=M=128, D=256, BD=1024):
    t  = einsum('bnd,dek->bnek', X, A)
    bi = einsum('bnek,bme->bnmk', t, Y)
    S  = einsum('bnmk,ok->bnmo', bi, W) + b        -> S[..., 0]  [B, N, M]

Algebraic collapse (exact reassociation):
    Aw[d, e] = sum_k A[d, e, k] * W[0, k]          # one streaming pass over A
    S[b]     = X[b] @ Aw @ Y[b].T + b              # tiny matmuls

Sharding: A is split over its first (d) axis across the 8 cores.  Each core
streams its 32 d-rows (16.75 MB as fp16), reduces them to Aw_c[32, 256],
computes its partial S_c = (X[:, :, rows_c] @ Aw_c) @ Y^T locally, and writes
S_c out.  The host sums the 8 partials and adds the bias — no device
collectives at all (the old AllGather-based design spent ~25us on the final
collective plus a ~48us serial tail).

Per-core pipeline (DMA-bound at ~350 GB/s for the 16.75 MB fp16 stream):
  - Host packs A_c as [kp=128, dl=32, kc=8, e=256] fp16 (k = kc*128 + kp), so
    k lives on SBUF partitions and each DMA group is 128 fully-contiguous
    per-partition runs.
  - DVE tensor_scalar multiplies each [128, r*256] block by W[kc*128+kp]
    (per-partition scalar -> eligible for the fast DVE modes).
  - PE reduces over the k partitions with an all-ones stationary operand
    (loaded once): psum[p, de] += sum_kp scr[kp, de]; 8 kc-blocks accumulate
    into one PSUM chunk.  All 128 psum rows are equal, so row 0 is the
    answer; tiny gpsimd DMAs stage it to DRAM (engines cannot write at a
    partition offset, so the d-partitioned Aw layout is rebuilt by one
    gather DMA instead).
  - Final: T = Aw_c^T X_c^T on PE, then S_c[b] = T^T Y_b^T, one fp32 copy,
    one 256 KB output DMA.
  - Numerics: products A*W would hit fp16 subnormals, so the host stages
    W*32 and X/32 (exact power-of-two rescale; S unchanged).
"""

import numpy as np

B, N, D, KD = 4, 128, 256, 1024
P = 128
C = 8                    # cores
DL = D // C              # 32 d-rows per core
KC = KD // P             # 8 k-blocks
GROUPS = [1, 1, 2, 4, 4, 4, 4, 4, 4, 2, 1, 1]    # d-rows per DMA (ramp both ends)
assert sum(GROUPS) == DL
XSCALE = 32.0            # host stages W*32 and X/32 to keep A*W out of fp16 subnormals

_cached = {}


def _build_program():
    import concourse.bass as bass
    import concourse.mybir as mybir
    import concourse.tile as tile
    from concourse import bacc

    fp32 = mybir.dt.float32
    fp16 = mybir.dt.float16

    nc = bacc.Bacc(
        "TRN2",
        target_bir_lowering=False,
        debug=False,
        num_devices=C,
    )

    # host-packed A shard: [kp, dl, kc, e] fp16, k = kc*128 + kp
    a_sh = nc.dram_tensor("a_sh", [P, DL, KC, D], fp16, kind="ExternalInput").ap()
    w_in = nc.dram_tensor("w_in", [P, KC], fp32, kind="ExternalInput").ap()   # W[kc*128+kp]*32
    xt_in = nc.dram_tensor("xt_in", [DL, B, N], fp16, kind="ExternalInput").ap()  # (X/32)^T local rows
    yt_in = nc.dram_tensor("yt_in", [D, B, N], fp16, kind="ExternalInput").ap()   # Y^T [e, b, m]
    out = nc.dram_tensor("out", [B, N, N], fp32, kind="ExternalOutput").ap()
    DEBUG = _cached.get("debug", False)
    if DEBUG:
        dbg_ones = nc.dram_tensor("dbg_ones", [P, P], fp16, kind="ExternalOutput").ap()
        dbg_scr0 = nc.dram_tensor("dbg_scr0", [P, KC, D], fp16, kind="ExternalOutput").ap()
        dbg_awflat = nc.dram_tensor("dbg_awflat", [1, DL * D], fp16, kind="ExternalOutput").ap()
        dbg_aw = nc.dram_tensor("dbg_aw", [DL, D], fp16, kind="ExternalOutput").ap()
        dbg_tT = nc.dram_tensor("dbg_tT", [P, 2, B, N], fp16, kind="ExternalOutput").ap()

    with tile.TileContext(nc) as tc:
        with (
            tc.tile_pool(name="apool", bufs=3) as apool,
            tc.tile_pool(name="spool", bufs=3) as spool,
            tc.tile_pool(name="sbuf", bufs=1) as sbuf,
            tc.tile_pool(name="pred", bufs=3, space="PSUM") as pred,
            tc.tile_pool(name="pfin", bufs=1, space="PSUM") as pfin,
            tc.tile_pool(name="dram", bufs=1, space="DRAM") as dram,
        ):
            # small inputs on the gpsimd SWDGE ring; sync ring stays on the A stream
            w_sb = sbuf.tile([P, KC], fp32)
            nc.gpsimd.dma_start(w_sb[:], w_in[:])
            xt_sb = sbuf.tile([DL, B, N], fp16)
            nc.gpsimd.dma_start(xt_sb[:], xt_in[:])
            yt_sb = sbuf.tile([P, 2, B, N], fp16)   # [e_lo, ec, b, m]
            nc.gpsimd.dma_start(yt_sb[:], yt_in.rearrange("(ec p) b m -> p ec b m", p=P))

            ones = sbuf.tile([P, P], fp16)
            nc.gpsimd.memset(ones[:], 1.0)

            aw_flat = sbuf.tile([1, DL * D], fp16)   # Aw staging on partition 0, (dl, e) order
            aw_sb = sbuf.tile([DL, D], fp16)

            r0 = 0
            for g, r in enumerate(GROUPS):
                at = apool.tile([P, 4, KC, D], fp16, tag="a", name=f"at{g}")
                nc.sync.dma_start(at[:, :r], a_sh[:, r0 : r0 + r])
                scr = spool.tile([P, 4, KC, D], fp16, tag="s", name=f"scr{g}")
                for kc in range(KC):
                    # scr = at * W[kc*128 + kp]  (per-partition scalar)
                    nc.vector.tensor_scalar_mul(
                        scr[:, :r, kc], at[:, :r, kc], w_sb[:, kc : kc + 1]
                    )
                if DEBUG and g == 0:
                    nc.sync.dma_start(dbg_scr0[:], scr[:, 0])
                # PE partition-reduce with all-ones stationary, 2 d-rows per chunk
                for c0 in range(0, r, 2):
                    cw = min(2, r - c0)
                    ps = pred.tile([P, 2 * D], fp32, tag="ps", name=f"ps{g}_{c0}")
                    for kc in range(KC):
                        nc.tensor.matmul(
                            ps[:, : cw * D],
                            lhsT=ones,
                            rhs=scr[:, c0 : c0 + cw, kc],
                            start=(kc == 0),
                            stop=(kc == KC - 1),
                        )
                    # all psum rows equal -> ACT stages row 0 (fp32->fp16 cast)
                    row = r0 + c0
                    nc.scalar.activation(
                        out=aw_flat[0:1, row * D : (row + cw) * D],
                        in_=ps[0:1, : cw * D],
                        func=mybir.ActivationFunctionType.Copy,
                    )
                r0 += r

            # rebuild Aw with d on partitions.  An SBUF->SBUF partition
            # scatter miscompiles on HW (sim-only correct), so bounce
            # through DRAM: contiguous store, then the standard scattered load.
            aw_dram = dram.tile([1, DL * D], fp16)
            nc.gpsimd.dma_start(aw_dram[:], aw_flat[:])
            nc.gpsimd.dma_start(
                aw_sb[:], aw_dram.rearrange("o (r e) -> (o r) e", r=DL)
            )
            psT = [pfin.tile([P, B * N], fp32, name=f"psT{ec}") for ec in range(2)]
            for ec in range(2):
                nc.tensor.matmul(
                    psT[ec],
                    lhsT=aw_sb[:, ec * P : (ec + 1) * P],
                    rhs=xt_sb[:],
                    start=True,
                    stop=True,
                )
            tT = sbuf.tile([P, 2, B, N], fp16)   # [e_lo, ec, b, n]
            for ec in range(2):
                nc.scalar.activation(
                    out=tT[:, ec], in_=psT[ec][:, :],
                    func=mybir.ActivationFunctionType.Copy,
                )
            psS = pfin.tile([P, B, N], fp32)     # [n, b, m]
            for b in range(B):
                for ec in range(2):
                    nc.tensor.matmul(
                        psS[:, b, :],
                        lhsT=tT[:, ec, b, :],
                        rhs=yt_sb[:, ec, b, :],
                        start=(ec == 0),
                        stop=(ec == 1),
                    )
            s_sb = sbuf.tile([P, B, N], fp32)
            nc.scalar.activation(
                out=s_sb[:], in_=psS[:, :, :],
                func=mybir.ActivationFunctionType.Copy,
            )
            nc.sync.dma_start(out.rearrange("b n m -> n b m"), s_sb[:])
            if DEBUG:
                nc.sync.dma_start(dbg_ones[:], ones[:])
                nc.sync.dma_start(dbg_awflat[:], aw_flat[:])
                nc.sync.dma_start(dbg_aw[:], aw_sb[:])
                nc.sync.dma_start(dbg_tT[:], tT[:])

    nc.compile()
    return nc


def _get_program():
    if "nc" not in _cached:
        _cached["nc"] = _build_program()
    return _cached["nc"]


def _run(X, Y, A, W, b, trace=False, **trace_kwargs):
    from concourse.bass_utils import run_bass_kernel_spmd

    nc = _get_program()

    A = np.asarray(A, dtype=np.float32)
    W = np.asarray(W, dtype=np.float32)
    X = np.asarray(X, dtype=np.float32)
    Y = np.asarray(Y, dtype=np.float32)

    # W * 32 laid out [kp, kc]; X / 32 transposed to [d, b, n] (exact 2^5 rescale)
    w_cols = np.ascontiguousarray(
        (W.reshape(KC, P) * np.float32(XSCALE)).T, dtype=np.float32
    )
    xt = np.ascontiguousarray(
        (X / np.float32(XSCALE)).transpose(2, 0, 1), dtype=np.float16
    )  # [d, b, n]
    yt = np.ascontiguousarray(Y.transpose(2, 0, 1), dtype=np.float16)  # [e, b, m]

    in_maps = []
    for c in range(C):
        rows = slice(c * DL, (c + 1) * DL)
        # [dl, e, k] -> [kp, dl, kc, e]
        a_perm = np.ascontiguousarray(
            A[rows].reshape(DL, D, KC, P).transpose(3, 0, 2, 1), dtype=np.float16
        )
        in_maps.append(
            {
                "a_sh": a_perm,
                "w_in": w_cols,
                "xt_in": np.ascontiguousarray(xt[rows]),
                "yt_in": yt,
            }
        )

    res = run_bass_kernel_spmd(nc, in_maps, list(range(C)), trace=trace, **trace_kwargs)
    # per-core outputs are partial sums over d; host unshard = sum + bias
    out = np.zeros((B, N, N), dtype=np.float32)
    for c in range(C):
        out += np.asarray(res.results[c]["out"], dtype=np.float32)
    out += np.float32(np.asarray(b).reshape(-1)[0])
    return out, res


def kernel(X, Y, A, W, b):
    out, _ = _run(X, Y, A, W, b, trace=False)
    return out
